# revision 18
# baseline (speedup 1.0000x reference)
"""Hawkes point-process log-likelihood on 8 Trainium2 NeuronCores.

Math: with sorted event times and beta_s = softplus(beta) > 0, every score in
the reference's (N,T,T) matrix is <= 0, so exp(logsumexp(scores)) is a plain
sum of decaying exponentials

    S_i = sum_{j<i} exp(-b*(t_i - t_j))  =  exp(-b*dt_i) * (S_{i-1} + 1),

an affine first-order recurrence. Per core (4 sequences of T=2048), times are
laid out as 128 partitions x 64 events (chunk (seq s, chunk c) per partition).
The within-chunk recurrence runs as ONE DVE tensor_tensor_scan
(state = dec*state + dec0), seeded per chunk with the cross-chunk carry

    R[c] = sum_{c'<c, same seq} exp(-b*(ref_c - ref_c')) * W[c'],
    W[c] = sum_j exp(+b*(t_j - ref_c)),   ref_c = first event time of chunk c,

computed with one 128x128 masked-exp matmul. Everything else (loglik,
compensator, dist) follows with per-partition broadcasts and tiny indicator
matmuls. All heavy inputs stay f32; total on-device work is O(N*T).
"""

import numpy as np

NCORES = 8
N, T = 32, 2048
SPC = N // NCORES          # sequences per core = 4
C = 64                     # events per chunk
NCH = T // C               # chunks per sequence = 32
P = SPC * NCH              # partitions used = 128
NEG_BIG = np.float32(-1e30)

_CACHE = {}


def _const_arrays():
    cp = np.arange(P)
    c = np.arange(P)
    same_seq = (cp[:, None] // NCH) == (c[None, :] // NCH)
    lower = cp[:, None] < c[None, :]
    offs = np.where(same_seq & lower, np.float32(0), NEG_BIG).astype(np.float32)

    s = np.arange(SPC)
    firstind = (cp[:, None] == s[None, :] * NCH).astype(np.float32)
    lastind = (cp[:, None] == s[None, :] * NCH + NCH - 1).astype(np.float32)
    seqind = ((cp[:, None] // NCH) == s[None, :]).astype(np.float32)
    consts = np.concatenate([offs, firstind, lastind, seqind], axis=1)  # [128,140]
    seqindt = seqind.T.copy()  # [4,128]
    return consts, seqindt


def _build_nc():
    import concourse.bacc as bacc
    import concourse.tile as tile
    from concourse import mybir
    from concourse.hw_specs import get_activation_tables
    import bass_rust as _bass_rust

    f32 = mybir.dt.float32
    AF = mybir.ActivationFunctionType
    OP = mybir.AluOpType

    class _Bacc(bacc.Bacc):
        # All our activations (Exp, Ln, Copy) live in the combined
        # natural_log_exp_and_others set; the default per-activation set
        # choice ping-pongs exp/ln sets costing ~1.3us per reload. Emptying
        # the other sets (list order preserved — act_func_set_id is the index
        # into act_info.json) forces a single table load.
        def insert_act_table_loads(self):
            tabs = get_activation_tables(self.m.arch)
            keep = [
                (k, (v if k == "natural_log_exp_and_others" else set()))
                for k, v in tabs.items()
            ]
            _bass_rust.insert_act_table_loads(self, keep)

    nc = _Bacc()
    t_d = nc.declare_dram_parameter("t", [P, C], f32, isOutput=False)
    par_d = nc.declare_dram_parameter("params", [128, 3], f32, isOutput=False)
    cst_d = nc.declare_dram_parameter("consts", [128, 140], f32, isOutput=False)
    sqt_d = nc.declare_dram_parameter("seqindt", [SPC, 128], f32, isOutput=False)
    lamb_d = nc.declare_dram_parameter("lamb_o", [P, C], f32, isOutput=True)
    dist_d = nc.declare_dram_parameter("dist_o", [P, C], f32, isOutput=True)
    nll_d = nc.declare_dram_parameter("nll_o", [SPC, 1], f32, isOutput=True)

    with tile.TileContext(nc) as tc:
        with (
            tc.tile_pool(name="main", bufs=1) as pool,
            tc.tile_pool(name="psum", bufs=1, space="PSUM") as psum,
        ):
            X = pool.tile([P, C], f32, tag="X")
            nc.sync.dma_start(out=X[:], in_=t_d[:])
            PAR = pool.tile([128, 3], f32, tag="PAR")
            nc.sync.dma_start(out=PAR[:], in_=par_d[:])
            CST = pool.tile([128, 140], f32, tag="CST")
            nc.sync.dma_start(out=CST[:], in_=cst_d[:])
            SQT = pool.tile([SPC, 128], f32, tag="SQT")
            nc.sync.dma_start(out=SQT[:], in_=sqt_d[:])
            # row of chunk reference times ref_c = t[c, 0], read strided from DRAM
            REFR = pool.tile([1, P], f32, tag="REFR")
            nc.sync.dma_start(out=REFR[:], in_=t_d[:, 0:1].rearrange("a b -> b a"))
            ONES1 = pool.tile([1, P], f32, tag="ONES1")
            nc.gpsimd.memset(ONES1[:], 1.0)
            # staging copies: this walrus build allows only ONE sync wait per
            # Matmult, so both operands of every matmul must be produced by a
            # single engine's sem domain.
            INDSV = pool.tile([128, 8], f32, tag="INDSV")  # first|last ind, DVE
            nc.vector.tensor_copy(INDSV[:], CST[:, 128:136])
            INDSA = pool.tile([128, 4], f32, tag="INDSA")  # seq ind, ACT
            nc.scalar.copy(INDSA[:], CST[:, 136:140])
            REFRC = pool.tile([1, P], f32, tag="REFRC")
            nc.gpsimd.tensor_copy(REFRC[:], REFR[:])
            SQTC = pool.tile([SPC, 128], f32, tag="SQTC")
            nc.scalar.copy(SQTC[:], SQT[:])
            EPS8 = pool.tile([128, 1], f32, tag="EPS8")
            nc.gpsimd.memset(EPS8[:], 1e-8)
            NEG1 = pool.tile([128, 1], f32, tag="NEG1")
            nc.gpsimd.memset(NEG1[:], -1.0)

            # softplus of (mu, alpha, beta), replicated on all partitions
            E1 = pool.tile([128, 3], f32, tag="E1")
            nc.scalar.activation(E1[:], PAR[:], AF.Exp)
            SP = pool.tile([128, 3], f32, tag="SP")
            nc.scalar.activation(SP[:], E1[:], AF.Ln, bias=1.0)
            MU, AL, BE = SP[:, 0:1], SP[:, 1:2], SP[:, 2:3]
            BNEG = pool.tile([128, 1], f32, tag="BNEG")
            nc.vector.tensor_scalar_mul(BNEG[:], SP[:, 2:3], -1.0)
            BINV = pool.tile([128, 1], f32, tag="BINV")
            nc.vector.reciprocal(BINV[:], SP[:, 2:3])
            AOB = pool.tile([128, 1], f32, tag="AOB")
            nc.vector.tensor_mul(AOB[:], BINV[:], SP[:, 1:2])

            # tau (chunk-relative times) and adjacent gaps
            TAU = pool.tile([P, C], f32, tag="TAU")
            nc.vector.tensor_scalar_sub(TAU[:], X[:], X[:, 0:1])
            DELT = pool.tile([P, C], f32, tag="DELT")
            nc.vector.memset(DELT[:, 0:1], 0.0)
            nc.vector.tensor_sub(DELT[:, 1:C], X[:, 1:C], X[:, 0 : C - 1])

            # dec = exp(-b*delta); dec0 = dec with column 0 zeroed
            DEC = pool.tile([P, C], f32, tag="DEC")
            nc.scalar.activation(DEC[:], DELT[:], AF.Exp, scale=BNEG[:, 0:1])
            DEC0 = pool.tile([P, C], f32, tag="DEC0")
            nc.gpsimd.memset(DEC0[:, 0:1], 0.0)
            nc.gpsimd.tensor_copy(DEC0[:, 1:C], DEC[:, 1:C])

            # V = exp(+b*tau) only for its row-sum W; U only at last column
            V = pool.tile([P, C], f32, tag="V")
            W = pool.tile([P, 1], f32, tag="W")
            nc.scalar.activation(V[:], TAU[:], AF.Exp, scale=BE, accum_out=W[:])
            U63 = pool.tile([P, 1], f32, tag="U63")
            nc.scalar.activation(U63[:], TAU[:, C - 1 : C], AF.Exp, scale=BNEG[:, 0:1])

            # cross-chunk carries: R = ET^T @ W with ET[c',c]=exp(-b(ref_c-ref_c'))
            RR = psum.tile([P, P], f32, tag="RR")
            nc.tensor.matmul(RR[:], ONES1[:], REFRC[:], start=True, stop=True)
            AM = pool.tile([P, P], f32, tag="AM")
            nc.vector.tensor_scalar(
                out=AM[:], in0=RR[:], scalar1=X[:, 0:1], scalar2=BNEG[:, 0:1],
                op0=OP.subtract, op1=OP.mult,
            )
            nc.vector.tensor_add(AM[:], AM[:], CST[:, 0:128])
            ET = pool.tile([P, P], f32, tag="ET")
            nc.scalar.activation(ET[:], AM[:], AF.Exp)
            Rp = psum.tile([P, 1], f32, tag="Rp")
            nc.tensor.matmul(Rp[:], ET[:], W[:], start=True, stop=True)

            # the Hawkes recurrence: one scan per chunk, seeded with carry R
            S = pool.tile([P, C], f32, tag="S")
            nc.vector.tensor_tensor_scan(
                out=S[:], data0=DEC[:], data1=DEC0[:], initial=Rp[:],
                op0=OP.mult, op1=OP.add,
            )
            LAM = pool.tile([P, C], f32, tag="LAM")
            nc.vector.tensor_scalar(
                out=LAM[:], in0=S[:], scalar1=AL, scalar2=MU,
                op0=OP.mult, op1=OP.add,
            )

            # loglik per sequence
            LLS = pool.tile([P, C], f32, tag="LLS")
            LLC = pool.tile([P, 1], f32, tag="LLC")
            nc.scalar.activation(
                LLS[:], LAM[:], AF.Ln, bias=EPS8[:, 0:1], accum_out=LLC[:]
            )
            LLK = psum.tile([SPC, 1], f32, tag="LLK")
            nc.tensor.matmul(LLK[:], INDSA[:], LLC[:], start=True, stop=True)

            # compensator pieces: t0, t_last, LK = U63*(R+W) at last chunk
            RW = pool.tile([P, 1], f32, tag="RW")
            nc.vector.tensor_add(RW[:], Rp[:], W[:])
            GAT = pool.tile([P, 3], f32, tag="GAT")
            nc.vector.tensor_copy(GAT[:, 0:1], X[:, C - 1 : C])
            nc.vector.tensor_mul(GAT[:, 1:2], U63[:], RW[:])
            nc.vector.tensor_copy(GAT[:, 2:3], X[:, 0:1])
            FIN1 = psum.tile([SPC, 1], f32, tag="FIN1")
            nc.tensor.matmul(FIN1[:], INDSV[:, 0:4], GAT[:, 2:3], start=True, stop=True)
            FIN2 = psum.tile([SPC, 2], f32, tag="FIN2")
            nc.tensor.matmul(FIN2[:], INDSV[:, 4:8], GAT[:, 0:2], start=True, stop=True)
            FINS = pool.tile([SPC, 3], f32, tag="FINS")
            nc.vector.tensor_copy(FINS[:, 0:1], FIN1[:])
            nc.vector.tensor_copy(FINS[:, 1:3], FIN2[:])

            TSP = pool.tile([SPC, 1], f32, tag="TSP")
            nc.vector.tensor_sub(TSP[:], FINS[:, 1:2], FINS[:, 0:1])
            C2 = pool.tile([SPC, 1], f32, tag="C2")
            nc.vector.tensor_scalar_mul(C2[:], FINS[:, 2:3], AOB[0:SPC, 0:1])
            COMP = pool.tile([SPC, 1], f32, tag="COMP")
            nc.vector.scalar_tensor_tensor(
                out=COMP[:], in0=TSP[:], scalar=MU[0:SPC, 0:1], in1=C2[:],
                op0=OP.mult, op1=OP.subtract,
            )
            OUT1 = pool.tile([SPC, 1], f32, tag="OUT1")
            nc.vector.tensor_sub(OUT1[:], LLK[:], COMP[:])
            nc.sync.dma_start(out=nll_d[:], in_=OUT1[:])

            ECO = pool.tile([SPC, 1], f32, tag="ECO")
            nc.scalar.activation(ECO[:], COMP[:], AF.Exp, scale=NEG1[0:SPC, 0:1])
            ECB = psum.tile([P, 1], f32, tag="ECB")
            nc.tensor.matmul(ECB[:], SQTC[:], ECO[:], start=True, stop=True)
            DIST = pool.tile([P, C], f32, tag="DIST")
            nc.vector.tensor_scalar_mul(DIST[:], LAM[:], ECB[:])

            nc.sync.dma_start(out=lamb_d[:], in_=LAM[:])
            nc.sync.dma_start(out=dist_d[:], in_=DIST[:])
    nc.compile()
    return nc


def _in_maps(input_time, mu, alpha, beta):
    t = np.ascontiguousarray(np.asarray(input_time, np.float32)[..., 0])  # (32,2048)
    params = np.empty((128, 3), np.float32)
    params[:, 0] = np.float32(np.asarray(mu).reshape(())[()])
    params[:, 1] = np.float32(np.asarray(alpha).reshape(())[()])
    params[:, 2] = np.float32(np.asarray(beta).reshape(())[()])
    consts, seqindt = _const_arrays()
    maps = []
    for k in range(NCORES):
        tc_ = np.ascontiguousarray(t[k * SPC : (k + 1) * SPC].reshape(P, C))
        maps.append({"t": tc_, "params": params, "consts": consts, "seqindt": seqindt})
    return maps


def _gather(results):
    ll = np.concatenate([r["nll_o"].reshape(SPC) for r in results]).astype(np.float32)
    dist = np.concatenate(
        [r["dist_o"].reshape(SPC, T) for r in results], axis=0
    ).astype(np.float32)
    lamb = np.concatenate(
        [r["lamb_o"].reshape(SPC, T) for r in results], axis=0
    ).astype(np.float32)
    return ll, dist, lamb


def run_spmd(input_time, mu, alpha, beta, **kw):
    """Build (cached), run on 8 cores, return (BassKernelResults, outputs)."""
    from concourse.bass_utils import run_bass_kernel_spmd

    if "nc" not in _CACHE:
        _CACHE["nc"] = _build_nc()
    nc = _CACHE["nc"]
    res = run_bass_kernel_spmd(
        nc, _in_maps(input_time, mu, alpha, beta), core_ids=list(range(NCORES)), **kw
    )
    return res, _gather(res.results)


def kernel(input_time, mu, alpha, beta):
    _, outs = run_spmd(input_time, mu, alpha, beta)
    return outs
